# revision 12
# baseline (speedup 1.0000x reference)
"""Trainium2 Bass kernel for nn_DFlashAttention_43774306681111.

Full-attention transformer block: QKV projection + per-head RMSNorm + neox
RoPE + GQA softmax attention (non-causal) + output projection.

Sharding (8 cores): 2-way data parallel over batch x 4-way tensor parallel
over heads. Core c handles batch c//4 and head group c%4 (q heads
4g..4g+3, kv head g). Each core computes a partial output [S, HID]
(its heads' contribution through Wo); the host sums the 4 partials per
batch. No device collectives.

Device layout: activations are kept transposed ([dim, token], dim on
partitions) so every matmul contracts on the partition axis:
  Q^T = Wq_tile^T @ X^T          (stationary Wq tile, moving X^T tile)
  S^T[k,q] = K^T_tile^T @ Q^T    (contraction d=128, one matmul per tile)
  softmax over k (= partitions): exp on ACT -> fp16 tiles; the 16 tiles of
    a block are pairwise-tree-summed on the DVE and a single ones-column
    matmul produces the denominators (instead of 16 PE matmuls per block)
  ctx^T[d,q] = V_tile^T @ expS^T (V stationary [k_tok, d])
  out[tok,hid] = ctxT_tile^T @ Wo
All matmul operands are fp16 (PSUM accumulation fp32); exp outputs are
scaled by 1/16 so fp16 row sums cannot overflow (the scale cancels in the
softmax normalization).

RoPE pairs (i, i+64) live on different partitions, so the half-swap is a
PE matmul with a 64-rotation permutation matrix (sign baked into the
host-built sin table); the RMSNorm sum-of-squares matmul uses a full ones
matrix so the result lands pre-broadcast across partitions. Both avoid
the DMA queues and the gpsimd partition_broadcast in the projection phase.

PE-stream discipline: the in-order PE queue is the bottleneck (~230ns per
512-row matmul sustained), so every matmul that depends on slow non-PE
work is emitted where that work is already finished: rmsnorm/rope tails
trickle through the next token block's stream, V transposes ride the
first attention block, the AV stagger carries across block boundaries,
per-block sum matmuls flush early in the following block, and Wo matmuls
for a finished query block spread through the next block's QK stream.
"""
import math
import numpy as np
from contextlib import ExitStack

import concourse.bass as bass
import concourse.tile as tile
from concourse import bacc, mybir
from concourse.bass_utils import run_bass_kernel_spmd

B, S, HID = 2, 2048, 2048
NH, NKV, D = 16, 4, 128
EPS = 1e-6
THETA = 1000000.0
SCALE = D ** -0.5

TP = 4                 # tensor-parallel groups (heads)
DP = 2                 # data-parallel over batch
HG = NH // TP          # q heads per core = 4
DQ = HG * D            # 512 q-proj cols per core
HALF = D // 2          # 64

F32 = mybir.dt.float32
F32R = mybir.dt.float32r
F16 = mybir.dt.float16
F16_NP = np.float16

HT = HID // 128        # 16 hid tiles
TBS = 512              # token block size
NTB = S // TBS         # 4 token blocks
KT = S // 128          # 16 key tiles
QB = S // TBS          # 4 query blocks
NDT = HG + 2           # 6 projection outputs: q0..q3, k, v^T

STAGGER = 3            # AV matmul lag behind QK/exp (carried across blocks)
LN16 = math.log(16.0)

_cache = {}


def _build(skip_w=False):
    nc = bacc.Bacc(None, target_bir_lowering=False, debug=False)

    # host pre-tiles everything to [128 partitions, ...] layouts
    xt = nc.dram_tensor("xt", [128, HT, S], F16, kind="ExternalInput")
    wq = nc.dram_tensor("wq", [128, HT, DQ], F16, kind="ExternalInput")
    wk = nc.dram_tensor("wk", [128, HT, D], F16, kind="ExternalInput")
    wv = nc.dram_tensor("wv", [128, HT, D], F16, kind="ExternalInput")
    wo = nc.dram_tensor("wo", [128, HG, HID], F16, kind="ExternalInput")
    cos2 = nc.dram_tensor("cos2", [D, S], F32, kind="ExternalInput")
    sin2 = nc.dram_tensor("sin2", [D, S], F32, kind="ExternalInput")
    qnw = nc.dram_tensor("qnw", [D, 1], F32, kind="ExternalInput")
    knw = nc.dram_tensor("knw", [D, 1], F32, kind="ExternalInput")
    iden_d = nc.dram_tensor("iden", [128, 128], F16, kind="ExternalInput")
    onesm_d = nc.dram_tensor("onesm", [128, 128], F16, kind="ExternalInput")
    perm_d = nc.dram_tensor("perm", [128, 128], F32R, kind="ExternalInput")
    ones_d = nc.dram_tensor("ones", [128, 1], F16, kind="ExternalInput")
    out = nc.dram_tensor("out", [S, HID], F32, kind="ExternalOutput")

    with tile.TileContext(nc) as tc, ExitStack() as ctx:
        const = ctx.enter_context(tc.tile_pool(name="const", bufs=1))
        big = ctx.enter_context(tc.tile_pool(name="big", bufs=1))
        blk = ctx.enter_context(tc.tile_pool(name="blk", bufs=8))
        outp = ctx.enter_context(tc.tile_pool(name="outp", bufs=3))
        scratch = ctx.enter_context(tc.tile_pool(name="scratch", bufs=2))
        rows = ctx.enter_context(tc.tile_pool(name="rows", bufs=2))
        psum = ctx.enter_context(tc.tile_pool(name="psum", bufs=1, space="PSUM"))

        # ---- constants ----
        ident = const.tile([128, 128], F16)
        nc.scalar.dma_start(out=ident[:], in_=iden_d[:])
        ones_mat = const.tile([128, 128], F16)
        nc.scalar.dma_start(out=ones_mat[:], in_=onesm_d[:])
        perm = const.tile([128, 128], F32R)
        nc.scalar.dma_start(out=perm[:], in_=perm_d[:])
        ones_col = const.tile([128, 1], F16)
        nc.scalar.dma_start(out=ones_col[:], in_=ones_d[:])
        eps_col = const.tile([128, 1], F32)
        nc.vector.memset(eps_col, EPS)
        nln16 = const.tile([128, 1], F32)
        nc.vector.memset(nln16, -LN16)
        qnw_sb = const.tile([D, 1], F32)
        nc.scalar.dma_start(out=qnw_sb[:], in_=qnw[:])
        knw_sb = const.tile([D, 1], F32)
        nc.scalar.dma_start(out=knw_sb[:], in_=knw[:])

        # ---- resident weights / big activations ----
        # wo reuses the wq slot ("bigw") after the last projection matmul
        wq_sb = big.tile([128, HT, DQ], F16, tag="bigw")
        wk_sb = big.tile([128, HT, D], F16, tag="wk")
        wv_sb = big.tile([128, HT, D], F16, tag="wv")
        cos_sb = big.tile([D, S], F32, tag="cos")
        sin_sb = big.tile([D, S], F32, tag="sin")

        qT = big.tile([D, HG, S], F16, tag="qT")         # Q^T per head
        kT = big.tile([D, S], F16, tag="kT")             # K^T
        vT = big.tile([D, S], F16, tag="vT")             # V^T (pre-transpose)
        v_sb = big.tile([128, KT, D], F16, tag="v")      # V [tok, d] tiles
        ctxT = big.tile([D, HG, S], F16, tag="ctxT")     # ctx^T per head

        # weight loads, grouped into few big DMAs (scalar queue)
        nc.scalar.dma_start(out=wq_sb[:, 0:4, :], in_=wq[:, 0:4, :])
        nc.scalar.dma_start(out=wk_sb[:], in_=wk[:])
        nc.scalar.dma_start(out=wv_sb[:], in_=wv[:])
        for g in range(1, 4):
            nc.scalar.dma_start(out=wq_sb[:, 4 * g:4 * g + 4, :],
                                in_=wq[:, 4 * g:4 * g + 4, :])
        nc.scalar.dma_start(out=cos_sb[:], in_=cos2[:])
        nc.scalar.dma_start(out=sin_sb[:], in_=sin2[:])

        def stationary(ht, dt):
            if dt < HG:
                return wq_sb[:, ht, dt * D:(dt + 1) * D]
            if dt == HG:
                return wk_sb[:, ht, :]
            return wv_sb[:, ht, :]

        # Deferred rmsnorm+rope tails. Each holds two PE matmuls (ssq
        # broadcast + rope half-swap); they are flushed at spread-out
        # points of the LATER PE stream so post-processing for token
        # block tb overlaps the projections of tb+1.
        pending_evict = []

        def flush_evict(k=1):
            for _ in range(min(k, len(pending_evict))):
                pending_evict.pop(0)()

        # ---- phase A: projections ----
        # prefetch all xt tiles for tb=0 (sync queue is xt-only in phase A)
        xt_tiles = {}
        for ht in range(HT):
            t = blk.tile([128, TBS], F16, tag="xt", bufs=20, name=f"xt_0_{ht}")
            nc.sync.dma_start(out=t[:], in_=xt[:, ht, 0:TBS])
            xt_tiles[(0, ht)] = t

        for tb in range(NTB):
            tsl = slice(tb * TBS, (tb + 1) * TBS)
            accs = [psum.tile([128, TBS], F32, tag=f"p{'ABCDEF'[dt]}",
                              name=f"acc_{tb}_{dt}") for dt in range(NDT)]
            for ht in range(HT):
                if tb + 1 < NTB:
                    t = blk.tile([128, TBS], F16, tag="xt", bufs=20,
                                 name=f"xt_{tb + 1}_{ht}")
                    nc.sync.dma_start(
                        out=t[:], in_=xt[:, ht, (tb + 1) * TBS:(tb + 2) * TBS])
                    xt_tiles[(tb + 1, ht)] = t
                if ht >= 2 and ht % 2 == 0:
                    flush_evict(1)  # previous tb's tails, one per ht pair
                xt_t = xt_tiles.pop((tb, ht))
                for dt in range(NDT):
                    nc.tensor.matmul(accs[dt][:], stationary(ht, dt), xt_t[:],
                                     start=(ht == 0), stop=(ht == HT - 1))
            for dt in [HG, NDT - 1, 0, 1, 2, 3]:
                acc = accs[dt]
                if dt == NDT - 1:
                    nc.scalar.copy(vT[:, tsl], acc[:])
                    continue
                w_ap = qnw_sb if dt < HG else knw_sb
                # single psum read on ACT frees the bank fast
                raw = scratch.tile([128, TBS], F32R, tag="raw", bufs=6,
                                   name=f"raw_{tb}_{dt}")
                nc.scalar.copy(raw[:], acc[:])
                if skip_w:
                    qn = raw  # norm weights are all-ones: skip the multiply
                else:
                    qn = scratch.tile([128, TBS], F32R, tag="qn", bufs=6,
                                      name=f"qn_{tb}_{dt}")
                    nc.scalar.activation(qn[:], raw[:],
                                         mybir.ActivationFunctionType.Copy,
                                         scale=w_ap[:])
                # q2 computed eagerly (from the unweighted raw) so the
                # deferred ssq matmul is ready when it lands on the PE
                q2 = scratch.tile([128, TBS], F16, tag="q2", bufs=6,
                                  name=f"q2_{tb}_{dt}")
                nc.scalar.square(q2[:], raw[:])

                def evict_tail(tb=tb, dt=dt, raw=raw, qn=qn, q2=q2, tsl=tsl):
                    # ssq broadcast: full ones matrix -> every partition
                    # holds the per-token sum of squares
                    ssq = psum.tile([128, TBS], F32, tag=["pG", "pH"][dt % 2],
                                    name=f"ssq_{tb}_{dt}")
                    nc.tensor.matmul(ssq[:], ones_mat[:], q2[:],
                                     start=True, stop=True)
                    # rope half-swap via permutation matmul (PE, no DMA)
                    xsw = psum.tile([128, TBS], F32, tag=["pH", "pG"][dt % 2],
                                    name=f"xsw_{tb}_{dt}")
                    nc.tensor.matmul(xsw[:], perm[:], qn[:],
                                     start=True, stop=True)
                    rstd = scratch.tile([128, TBS], F32, tag="rstd", bufs=4,
                                        name=f"rstd_{tb}_{dt}")
                    nc.scalar.activation(rstd[:], ssq[:],
                                         mybir.ActivationFunctionType.Sqrt,
                                         scale=1.0 / D, bias=eps_col[:])
                    nc.vector.reciprocal_approx_fast(out=rstd[:], in_=rstd[:])
                    tmp = scratch.tile([128, TBS], F32, tag="tmp", bufs=2,
                                       name=f"tmp_{tb}_{dt}")
                    nc.vector.tensor_mul(tmp[:], qn[:], cos_sb[:, tsl])
                    sv = scratch.tile([128, TBS], F32, tag="sv", bufs=2,
                                      name=f"sv_{tb}_{dt}")
                    nc.vector.tensor_mul(sv[:], xsw[:], sin_sb[:, tsl])
                    qro = scratch.tile([128, TBS], F32, tag="qro", bufs=2,
                                       name=f"qro_{tb}_{dt}")
                    nc.gpsimd.tensor_add(qro[:], tmp[:], sv[:])
                    dest = qT[:, dt, tsl] if dt < HG else kT[:, tsl]
                    nc.vector.tensor_mul(dest, qro[:], rstd[:])
                pending_evict.append(evict_tail)

        # wo loads overlap the first attention blocks ("bigw" frees after
        # the last projection matmul)
        wo_sb = big.tile([128, HG, HID], F16, tag="bigw")
        for ct in range(HG):
            nc.sync.dma_start(out=wo_sb[:, ct, :], in_=wo[:, ct, :])

        # V transposes ride the first attention block's PE stream
        pending_tp = list(range(KT))

        def flush_tp(k=1):
            for _ in range(min(k, len(pending_tp))):
                kt0 = pending_tp.pop(0)
                tp = psum.tile([128, 128], F16, tag=["pD", "pG"][kt0 % 2],
                               name=f"tp_{kt0}")
                nc.tensor.transpose(tp[:], vT[:, kt0 * 128:(kt0 + 1) * 128],
                                    ident[:])
                if kt0 % 2 == 0:
                    nc.vector.tensor_copy(v_sb[:, kt0, :], tp[:])
                else:
                    nc.scalar.copy(v_sb[:, kt0, :], tp[:])

        # ---- phase B: attention (qb-major) with Wo folded in ----
        pending_wo = []

        def emit_wo(qb):
            thunks = []
            for tt in range(qb * (TBS // 128), (qb + 1) * (TBS // 128)):
                for hc in range(HID // TBS):
                    def thunk(tt=tt, hc=hc):
                        o_ps = psum.tile([128, TBS], F32,
                                         tag=f"p{'EF'[(tt * 4 + hc) % 2]}",
                                         name=f"o_{tt}_{hc}")
                        for ct in range(HG):
                            nc.tensor.matmul(
                                o_ps[:],
                                ctxT[:, ct, tt * 128:(tt + 1) * 128],
                                wo_sb[:, ct, hc * TBS:(hc + 1) * TBS],
                                start=(ct == 0), stop=(ct == HG - 1))
                        o_sb = outp.tile([128, TBS], F32, tag="osb",
                                         name=f"osb_{tt}_{hc}")
                        if (tt * 4 + hc) % 2 == 0:
                            nc.scalar.copy(o_sb[:], o_ps[:])
                        else:
                            nc.vector.tensor_copy(o_sb[:], o_ps[:])
                        nc.sync.dma_start(
                            out=out[tt * 128:(tt + 1) * 128,
                                    hc * TBS:(hc + 1) * TBS],
                            in_=o_sb[:])
                    thunks.append(thunk)
            return thunks

        def flush_wo(k):
            for _ in range(min(k, len(pending_wo))):
                pending_wo.pop(0)()

        # cross-block AV stagger; entries: (kt, e, ctx_ps, blk)
        pend = []
        norm_jobs = {}
        sum_jobs = {}

        def flush_av():
            kt0, e0, c_ps, bi = pend.pop(0)
            nc.tensor.matmul(c_ps[:], v_sb[:, kt0, :], e0[:],
                             start=(kt0 == 0), stop=(kt0 == KT - 1))
            if kt0 == KT - 1 and bi in norm_jobs:
                norm_jobs.pop(bi)()

        for qb in range(QB):
            qsl = slice(qb * TBS, (qb + 1) * TBS)
            for h in range(HG):
                blk_i = qb * HG + h
                ctx_ps = psum.tile([128, TBS], F32,
                                   tag=f"p{'CD'[blk_i % 2]}",
                                   name=f"ctx_{h}_{qb}")
                # fp16 exp tiles and their pairwise-reduction tree (DVE)
                t1 = [None] * 8
                t2 = [None] * 4
                t3 = [None] * 2
                es = [None]
                etiles = [None] * KT

                for kt in range(KT):
                    g = blk_i * KT + kt
                    if kt == 2 and blk_i - 1 in sum_jobs:
                        sum_jobs.pop(blk_i - 1)()
                    # block 0 keeps pH/pG free for the tails and transposes
                    s_tag = "AB"[g % 2] if blk_i == 0 else "ABH"[g % 3]
                    s_ps = psum.tile([128, TBS], F32, tag=f"p{s_tag}",
                                     name=f"s_{h}_{qb}_{kt}")
                    nc.tensor.matmul(s_ps[:], kT[:, kt * 128:(kt + 1) * 128],
                                     qT[:, h, qsl], start=True, stop=True)
                    e = blk.tile([128, TBS], F16, tag="blk",
                                 name=f"e_{h}_{qb}_{kt}")
                    nc.scalar.activation(e[:], s_ps[:],
                                         mybir.ActivationFunctionType.Exp,
                                         scale=SCALE, bias=nln16[:])
                    etiles[kt] = e
                    if kt % 2 == 1:
                        j = kt // 2
                        t1[j] = scratch.tile([128, TBS], F16, tag="t1", bufs=8,
                                             name=f"t1_{blk_i}_{j}")
                        nc.vector.tensor_add(t1[j][:], etiles[kt - 1][:], e[:])
                    if kt % 4 == 3:
                        j = kt // 4
                        t2[j] = scratch.tile([128, TBS], F16, tag="t2", bufs=4,
                                             name=f"t2_{blk_i}_{j}")
                        nc.vector.tensor_add(t2[j][:], t1[2 * j][:],
                                             t1[2 * j + 1][:])
                    if kt % 8 == 7:
                        j = kt // 8
                        t3[j] = scratch.tile([128, TBS], F16, tag="t3", bufs=2,
                                             name=f"t3_{blk_i}_{j}")
                        nc.vector.tensor_add(t3[j][:], t2[2 * j][:],
                                             t2[2 * j + 1][:])
                    if kt == KT - 1:
                        es[0] = scratch.tile([128, TBS], F16, tag="es", bufs=2,
                                             name=f"es_{blk_i}")
                        nc.vector.tensor_add(es[0][:], t3[0][:], t3[1][:])
                    pend.append((kt, e, ctx_ps, blk_i))
                    if len(pend) > STAGGER:
                        flush_av()
                    # spread deferred work through the first block's stream
                    if blk_i == 0:
                        if kt % 4 == 0 or kt == 14:
                            flush_evict(1)
                        if kt >= 2:
                            flush_tp(2)
                    if qb > 0 and kt in (5, 9):
                        flush_wo(2)

                def sum_job(blk_i=blk_i, es=es):
                    sum_ps = psum.tile([1, TBS], F32, tag="pG", bufs=1,
                                       name=f"sum_{blk_i}")
                    nc.tensor.matmul(sum_ps[:], ones_col[:], es[0][:],
                                     start=True, stop=True)
                    recip = rows.tile([1, TBS], F32, tag="recip",
                                      name=f"recip_{blk_i}")
                    nc.vector.reciprocal_approx_fast(out=recip[:],
                                                     in_=sum_ps[:])
                    recipb = scratch.tile([128, TBS], F32, tag="bcast",
                                          bufs=2, name=f"recipb_{blk_i}")
                    nc.gpsimd.partition_broadcast(recipb[:], recip[:])
                    sum_jobs[blk_i] = ("done", recipb)
                    return recipb
                sum_jobs[blk_i] = lambda blk_i=blk_i, es=es: sum_job(blk_i, es)

                def norm_job(h=h, qb=qb, qsl=qsl, ctx_ps=ctx_ps, blk_i=blk_i):
                    ent = sum_jobs.pop(blk_i, None)
                    if callable(ent):
                        recipb = ent()
                        sum_jobs.pop(blk_i, None)
                    else:
                        recipb = ent[1]
                    nc.vector.tensor_mul(ctxT[:, h, qsl], ctx_ps[:],
                                         recipb[:])
                norm_jobs[blk_i] = norm_job
            pending_wo.extend(emit_wo(qb))

        while pend:
            flush_av()
        for i in sorted(list(norm_jobs)):
            norm_jobs.pop(i)()
        flush_wo(len(pending_wo))

    nc.compile()
    return nc


def _prep_inputs(hidden_states, positions, Wq, Wk, Wv, Wo, q_norm_w, k_norm_w):
    hidden_states = np.asarray(hidden_states, dtype=np.float32)
    positions = np.asarray(positions)
    Wq = np.asarray(Wq, dtype=np.float32)
    Wk = np.asarray(Wk, dtype=np.float32)
    Wv = np.asarray(Wv, dtype=np.float32)
    Wo = np.asarray(Wo, dtype=np.float32)
    q_norm_w = np.asarray(q_norm_w, dtype=np.float32)
    k_norm_w = np.asarray(k_norm_w, dtype=np.float32)

    inv_freq = THETA ** (-np.arange(HALF, dtype=np.float32) / HALF)
    perm_m = np.zeros((128, 128), dtype=np.float32)
    perm_m[np.arange(128), (np.arange(128) + HALF) % 128] = 1

    def tile_p(a, nt, w):
        # [nt*128, w] -> [128, nt, w]
        return np.ascontiguousarray(
            a.reshape(nt, 128, w).transpose(1, 0, 2)).astype(F16_NP)

    in_maps = []
    for c in range(DP * TP):
        b, g = divmod(c, TP)
        freqs = positions[b].astype(np.float32)[:, None] * inv_freq[None, :]
        cos = np.cos(freqs).T.astype(np.float32)      # [64, S]
        sin = np.sin(freqs).T.astype(np.float32)
        cos2 = np.ascontiguousarray(np.concatenate([cos, cos], axis=0))
        sin2 = np.ascontiguousarray(np.concatenate([-sin, sin], axis=0))
        in_maps.append({
            "xt": tile_p(hidden_states[b].T, HT, S),
            "wq": tile_p(Wq[:, g * DQ:(g + 1) * DQ], HT, DQ),
            "wk": tile_p(Wk[:, g * D:(g + 1) * D], HT, D),
            "wv": tile_p(Wv[:, g * D:(g + 1) * D], HT, D),
            "wo": tile_p(Wo[g * DQ:(g + 1) * DQ, :], HG, HID),
            "cos2": cos2,
            "sin2": sin2,
            "qnw": np.ascontiguousarray(q_norm_w[:, None]),
            "knw": np.ascontiguousarray(k_norm_w[:, None]),
            "iden": np.eye(128, dtype=F16_NP),
            "onesm": np.ones((128, 128), dtype=F16_NP),
            "perm": perm_m,
            "ones": np.ones((128, 1), dtype=F16_NP),
        })
    return in_maps


def _run(inputs, trace=False):
    skip_w = bool(np.allclose(inputs["q_norm_w"], 1.0)
                  and np.allclose(inputs["k_norm_w"], 1.0))
    key = ("nc", skip_w)
    if key not in _cache:
        _cache[key] = _build(skip_w)
    nc = _cache[key]
    in_maps = _prep_inputs(**inputs)
    res = run_bass_kernel_spmd(nc, in_maps, core_ids=list(range(DP * TP)),
                               trace=trace)
    out = np.zeros((B, S, HID), dtype=np.float32)
    for c in range(DP * TP):
        out[c // TP] += res.results[c]["out"]
    return out, res


def kernel(**inputs):
    out, _ = _run(inputs, trace=False)
    return out


# revision 25
# speedup vs baseline: 1.0238x; 1.0238x over previous
"""Trainium2 Bass kernel for nn_DFlashAttention_43774306681111.

Full-attention transformer block: QKV projection + per-head RMSNorm + neox
RoPE + GQA softmax attention (non-causal) + output projection.

Sharding (8 cores): 2-way data parallel over batch x 4-way tensor parallel
over heads. Core c handles batch c//4 and head group c%4 (q heads
4g..4g+3, kv head g). Each core computes a partial output [S, HID]
(its heads' contribution through Wo); the host sums the 4 partials per
batch. No device collectives.

Device layout: activations are kept transposed ([dim, token], dim on
partitions) so every matmul contracts on the partition axis:
  Q^T = Wq_tile^T @ X^T          (stationary Wq tile, moving X^T tile)
  S^T[k,q] = K^T_tile^T @ Q^T    (contraction d=128, one matmul per tile)
  softmax over k (= partitions): exp on ACT -> fp16 tiles; the 16 tiles of
    a block are pairwise-tree-summed on the DVE and a single ones-column
    matmul produces the denominators (instead of 16 PE matmuls per block)
  ctx^T[d,q] = V_tile^T @ expS^T (V stationary [k_tok, d])
  out[tok,hid] = ctxT_tile^T @ Wo
All matmul operands are fp16 (PSUM accumulation fp32); exp outputs are
scaled by 1/16 so fp16 row sums cannot overflow (the scale cancels in the
softmax normalization).

RoPE pairs (i, i+64) live on different partitions, so the half-swap is a
PE matmul with a 64-rotation permutation matrix (sign baked into the
host-built sin table); the RMSNorm sum-of-squares matmul uses a full ones
matrix so the result lands pre-broadcast across partitions. Both avoid
the DMA queues and the gpsimd partition_broadcast in the projection phase.

PE-stream discipline: the in-order PE queue is the bottleneck (~230ns per
512-row matmul sustained), so every matmul that depends on slow non-PE
work is emitted where that work is already finished: rmsnorm/rope tails
trickle through the next token block's stream, V transposes ride the
first attention block, the AV stagger carries across block boundaries,
per-block sum matmuls flush early in the following block, and Wo matmuls
for a finished query block spread through the next block's QK stream.
"""
import math
import numpy as np
from contextlib import ExitStack

import concourse.bass as bass
import concourse.tile as tile
from concourse import bacc, mybir
from concourse.bass_utils import run_bass_kernel_spmd

B, S, HID = 2, 2048, 2048
NH, NKV, D = 16, 4, 128
EPS = 1e-6
THETA = 1000000.0
SCALE = D ** -0.5

TP = 4                 # tensor-parallel groups (heads)
DP = 2                 # data-parallel over batch
HG = NH // TP          # q heads per core = 4
DQ = HG * D            # 512 q-proj cols per core
HALF = D // 2          # 64

F32 = mybir.dt.float32
F32R = mybir.dt.float32r
F16 = mybir.dt.float16
F16_NP = np.float16

HT = HID // 128        # 16 hid tiles
TBS = 512              # token block size
NTB = S // TBS         # 4 token blocks
KT = S // 128          # 16 key tiles
QB = S // TBS          # 4 query blocks
NDT = HG + 2           # 6 projection outputs: q0..q3, k, v^T

STAGGER = 3            # AV matmul lag behind QK/exp (carried across blocks)
LN16 = math.log(16.0)

_cache = {}


def _build(skip_w=False):
    nc = bacc.Bacc(None, target_bir_lowering=False, debug=False)

    # host pre-tiles everything to [128 partitions, ...] layouts
    xt = nc.dram_tensor("xt", [128, HT, S], F16, kind="ExternalInput")
    wq = nc.dram_tensor("wq", [128, HT, DQ], F16, kind="ExternalInput")
    wk = nc.dram_tensor("wk", [128, HT, D], F16, kind="ExternalInput")
    wv = nc.dram_tensor("wv", [128, HT, D], F16, kind="ExternalInput")
    wo = nc.dram_tensor("wo", [128, HG, HID], F16, kind="ExternalInput")
    cos2 = nc.dram_tensor("cos2", [D, S], F32, kind="ExternalInput")
    sin2 = nc.dram_tensor("sin2", [D, S], F32, kind="ExternalInput")
    qnw = nc.dram_tensor("qnw", [D, 1], F32, kind="ExternalInput")
    knw = nc.dram_tensor("knw", [D, 1], F32, kind="ExternalInput")
    iden_d = nc.dram_tensor("iden", [128, 128], F16, kind="ExternalInput")
    onesm_d = nc.dram_tensor("onesm", [128, 128], F16, kind="ExternalInput")
    perm_d = nc.dram_tensor("perm", [128, 128], F32R, kind="ExternalInput")
    ones_d = nc.dram_tensor("ones", [128, 1], F16, kind="ExternalInput")
    out = nc.dram_tensor("out", [S, HID], F32, kind="ExternalOutput")

    with tile.TileContext(nc) as tc, ExitStack() as ctx:
        const = ctx.enter_context(tc.tile_pool(name="const", bufs=1))
        big = ctx.enter_context(tc.tile_pool(name="big", bufs=1))
        blk = ctx.enter_context(tc.tile_pool(name="blk", bufs=8))
        outp = ctx.enter_context(tc.tile_pool(name="outp", bufs=3))
        scratch = ctx.enter_context(tc.tile_pool(name="scratch", bufs=2))
        rows = ctx.enter_context(tc.tile_pool(name="rows", bufs=2))
        psum = ctx.enter_context(tc.tile_pool(name="psum", bufs=1, space="PSUM"))

        # ---- resident weights / big activations ----
        # wo reuses the wq slot ("bigw") after the last projection matmul
        wq_sb = big.tile([128, HT, DQ], F16, tag="bigw")
        wk_sb = big.tile([128, HT, D], F16, tag="wk")
        wv_sb = big.tile([128, HT, D], F16, tag="wv")
        cos_sb = big.tile([D, S], F32, tag="cos")
        sin_sb = big.tile([D, S], F32, tag="sin")

        # first projection tiles load first so the PE starts ASAP
        nc.scalar.dma_start(out=wq_sb[:, 0:2, :], in_=wq[:, 0:2, :])
        nc.scalar.dma_start(out=wk_sb[:, 0:2, :], in_=wk[:, 0:2, :])
        nc.scalar.dma_start(out=wv_sb[:, 0:2, :], in_=wv[:, 0:2, :])

        qT = big.tile([D, HG, S], F16, tag="qT")         # Q^T per head
        kT = big.tile([D, S], F16, tag="kT")             # K^T
        vT = big.tile([D, S], F16, tag="vT")             # V^T (pre-transpose)
        v_sb = big.tile([128, KT, D], F16, tag="v")      # V [tok, d] tiles
        ctxT = big.tile([D, HG, S], F16, tag="ctxT")     # ctx^T per head

        # remaining weight loads, grouped into few big DMAs (scalar queue)
        nc.scalar.dma_start(out=wq_sb[:, 2:6, :], in_=wq[:, 2:6, :])
        nc.scalar.dma_start(out=wk_sb[:, 2:HT, :], in_=wk[:, 2:HT, :])
        nc.scalar.dma_start(out=wv_sb[:, 2:HT, :], in_=wv[:, 2:HT, :])
        nc.scalar.dma_start(out=wq_sb[:, 6:11, :], in_=wq[:, 6:11, :])
        nc.scalar.dma_start(out=wq_sb[:, 11:HT, :], in_=wq[:, 11:HT, :])
        nc.scalar.dma_start(out=cos_sb[:], in_=cos2[:])
        nc.scalar.dma_start(out=sin_sb[:], in_=sin2[:])

        # ---- constants (needed from the first tail flush, ~20us in) ----
        ident = const.tile([128, 128], F16)
        nc.scalar.dma_start(out=ident[:], in_=iden_d[:])
        ones_mat = const.tile([128, 128], F16)
        nc.scalar.dma_start(out=ones_mat[:], in_=onesm_d[:])
        perm = const.tile([128, 128], F32R)
        nc.scalar.dma_start(out=perm[:], in_=perm_d[:])
        ones_col = const.tile([128, 1], F16)
        nc.scalar.dma_start(out=ones_col[:], in_=ones_d[:])
        eps_col = const.tile([128, 1], F32)
        nc.vector.memset(eps_col, EPS)
        nln16 = const.tile([128, 1], F32)
        nc.vector.memset(nln16, -LN16)
        qnw_sb = const.tile([D, 1], F32)
        nc.scalar.dma_start(out=qnw_sb[:], in_=qnw[:])
        knw_sb = const.tile([D, 1], F32)
        nc.scalar.dma_start(out=knw_sb[:], in_=knw[:])

        def stationary(ht, dt):
            if dt < HG:
                return wq_sb[:, ht, dt * D:(dt + 1) * D]
            if dt == HG:
                return wk_sb[:, ht, :]
            return wv_sb[:, ht, :]

        # Deferred rmsnorm+rope tails. Each holds two PE matmuls (ssq
        # broadcast + rope half-swap); they are flushed at spread-out
        # points of the LATER PE stream so post-processing for token
        # block tb overlaps the projections of tb+1.
        pending_evict = []
        evict_n = [0]

        def flush_evict(k=1):
            for _ in range(min(k, len(pending_evict))):
                n = evict_n[0]
                evict_n[0] = n + 1
                pair = [("pG", "pH"), ("pH", "pG")][n % 2]
                front, back = pending_evict.pop(0)
                rstd = front(pair[0])
                back(pair[1], rstd)

        # ---- phase A: projections ----
        # prefetch all xt tiles for tb=0 (sync queue is xt-only in phase A)
        xt_tiles = {}
        for ht in range(HT):
            t = blk.tile([128, TBS], F16, tag="xt", bufs=20, name=f"xt_0_{ht}")
            nc.sync.dma_start(out=t[:], in_=xt[:, ht, 0:TBS])
            xt_tiles[(0, ht)] = t

        for tb in range(NTB):
            tsl = slice(tb * TBS, (tb + 1) * TBS)
            accs = [psum.tile([128, TBS], F32, tag=f"p{'ABCDEF'[dt]}",
                              name=f"acc_{tb}_{dt}") for dt in range(NDT)]
            for ht in range(HT):
                if tb + 1 < NTB:
                    t = blk.tile([128, TBS], F16, tag="xt", bufs=20,
                                 name=f"xt_{tb + 1}_{ht}")
                    nc.sync.dma_start(
                        out=t[:], in_=xt[:, ht, (tb + 1) * TBS:(tb + 2) * TBS])
                    xt_tiles[(tb + 1, ht)] = t
                if ht >= 2 and ht % 2 == 0:
                    flush_evict(1)  # previous tb's tails, one per ht pair
                xt_t = xt_tiles.pop((tb, ht))
                for dt in range(NDT):
                    nc.tensor.matmul(accs[dt][:], stationary(ht, dt), xt_t[:],
                                     start=(ht == 0), stop=(ht == HT - 1))
            last_tb = tb == NTB - 1
            for di, dt in enumerate([HG, NDT - 1, 0, 1, 2, 3]):
                acc = accs[dt]
                if dt == NDT - 1:
                    nc.scalar.copy(vT[:, tsl], acc[:])
                    continue
                w_ap = qnw_sb if dt < HG else knw_sb
                # single psum read frees the bank fast (the last token
                # block spreads these across engines: the ACT queue must
                # stay short so the first attention exp starts early)
                raw = scratch.tile([128, TBS], F32R, tag="raw", bufs=6,
                                   name=f"raw_{tb}_{dt}")
                if last_tb and dt in (2, 3):
                    nc.vector.tensor_copy(raw[:], acc[:])
                else:
                    nc.scalar.copy(raw[:], acc[:])
                if skip_w:
                    qn = raw  # norm weights are all-ones: skip the multiply
                else:
                    qn = scratch.tile([128, TBS], F32R, tag="qn", bufs=6,
                                      name=f"qn_{tb}_{dt}")
                    nc.scalar.activation(qn[:], raw[:],
                                         mybir.ActivationFunctionType.Copy,
                                         scale=w_ap[:])
                # q2 computed eagerly (from the unweighted raw) so the
                # deferred ssq matmul is ready when it lands on the PE
                q2 = scratch.tile([128, TBS], F16, tag="q2", bufs=6,
                                  name=f"q2_{tb}_{dt}")
                if last_tb:
                    eng = nc.gpsimd if dt in (HG, 0, 1) else nc.vector
                    eng.tensor_mul(q2[:], raw[:], raw[:])
                else:
                    nc.scalar.square(q2[:], raw[:])

                def tail_front(bank, tb=tb, dt=dt, q2=q2):
                    # ssq broadcast: full ones matrix -> every partition
                    # holds the per-token sum of squares
                    ssq = psum.tile([128, TBS], F32, tag=bank,
                                    name=f"ssq_{tb}_{dt}")
                    nc.tensor.matmul(ssq[:], ones_mat[:], q2[:],
                                     start=True, stop=True)
                    rstd = scratch.tile([128, TBS], F32, tag="rstd", bufs=4,
                                        name=f"rstd_{tb}_{dt}")
                    nc.scalar.activation(rstd[:], ssq[:],
                                         mybir.ActivationFunctionType.Sqrt,
                                         scale=1.0 / D, bias=eps_col[:])
                    nc.vector.reciprocal_approx_fast(out=rstd[:], in_=rstd[:])
                    return rstd

                def tail_back(bank, rstd, tb=tb, dt=dt, qn=qn, tsl=tsl,
                              last_tb=last_tb):
                    # rope half-swap via permutation matmul (PE, no DMA)
                    xsw = psum.tile([128, TBS], F32, tag=bank,
                                    name=f"xsw_{tb}_{dt}")
                    nc.tensor.matmul(xsw[:], perm[:], qn[:],
                                     start=True, stop=True)
                    tmp = scratch.tile([128, TBS], F32, tag="tmp", bufs=2,
                                       name=f"tmp_{tb}_{dt}")
                    if last_tb:
                        nc.gpsimd.tensor_mul(tmp[:], qn[:], cos_sb[:, tsl])
                    else:
                        nc.vector.tensor_mul(tmp[:], qn[:], cos_sb[:, tsl])
                    sv = scratch.tile([128, TBS], F32, tag="sv", bufs=2,
                                      name=f"sv_{tb}_{dt}")
                    nc.vector.tensor_mul(sv[:], xsw[:], sin_sb[:, tsl])
                    qro = scratch.tile([128, TBS], F32, tag="qro", bufs=2,
                                       name=f"qro_{tb}_{dt}")
                    nc.gpsimd.tensor_add(qro[:], tmp[:], sv[:])
                    dest = qT[:, dt, tsl] if dt < HG else kT[:, tsl]
                    nc.vector.tensor_mul(dest, qro[:], rstd[:])
                pending_evict.append((tail_front, tail_back))

        # wo loads overlap the first attention blocks ("bigw" frees after
        # the last projection matmul)
        wo_sb = big.tile([128, HG, HID], F16, tag="bigw")
        for ct in range(HG):
            nc.sync.dma_start(out=wo_sb[:, ct, :], in_=wo[:, ct, :])

        # Drain the last token block's tails and do all V transposes BEFORE
        # the first exp: Sqrt and Exp live in different ACT table sets, so
        # interleaving them costs a 1283ns ACT_TABLE_LOAD per switch. The
        # transposes fill the PE while each tail's ACT/DVE chain drains.
        tp_seq = list(range(KT))

        def emit_tp(k):
            for _ in range(min(k, len(tp_seq))):
                kt0 = tp_seq.pop(0)
                tp = psum.tile([128, 128], F16, tag=["pC", "pD"][kt0 % 2],
                               name=f"tp_{kt0}")
                nc.tensor.transpose(tp[:], vT[:, kt0 * 128:(kt0 + 1) * 128],
                                    ident[:])
                if kt0 % 2 == 0:
                    nc.vector.tensor_copy(v_sb[:, kt0, :], tp[:])
                else:
                    nc.scalar.copy(v_sb[:, kt0, :], tp[:])

        while pending_evict:
            flush_evict(1)
            emit_tp(3)
        emit_tp(KT)

        # ---- phase B: attention (qb-major) with Wo folded in ----
        pending_wo = []

        def emit_wo(qb):
            thunks = []
            for tt in range(qb * (TBS // 128), (qb + 1) * (TBS // 128)):
                for hc in range(HID // TBS):
                    def thunk(tt=tt, hc=hc):
                        o_ps = psum.tile([128, TBS], F32,
                                         tag=f"p{'EF'[(tt * 4 + hc) % 2]}",
                                         name=f"o_{tt}_{hc}")
                        for ct in range(HG):
                            nc.tensor.matmul(
                                o_ps[:],
                                ctxT[:, ct, tt * 128:(tt + 1) * 128],
                                wo_sb[:, ct, hc * TBS:(hc + 1) * TBS],
                                start=(ct == 0), stop=(ct == HG - 1))
                        o_sb = outp.tile([128, TBS], F32, tag="osb",
                                         name=f"osb_{tt}_{hc}")
                        if (tt * 4 + hc) % 2 == 0:
                            nc.scalar.copy(o_sb[:], o_ps[:])
                        else:
                            nc.vector.tensor_copy(o_sb[:], o_ps[:])
                        nc.sync.dma_start(
                            out=out[tt * 128:(tt + 1) * 128,
                                    hc * TBS:(hc + 1) * TBS],
                            in_=o_sb[:])
                    thunks.append(thunk)
            return thunks

        def flush_wo(k):
            for _ in range(min(k, len(pending_wo))):
                pending_wo.pop(0)()

        # cross-block AV stagger; entries: (kt, e, ctx_ps, blk)
        pend = []
        norm_jobs = {}
        sum_jobs = {}

        def flush_av():
            kt0, e0, c_ps, bi = pend.pop(0)
            nc.tensor.matmul(c_ps[:], v_sb[:, kt0, :], e0[:],
                             start=(kt0 == 0), stop=(kt0 == KT - 1))
            if kt0 == KT - 1 and bi in norm_jobs:
                norm_jobs.pop(bi)()

        for qb in range(QB):
            qsl = slice(qb * TBS, (qb + 1) * TBS)
            for h in range(HG):
                blk_i = qb * HG + h
                ctx_ps = psum.tile([128, TBS], F32,
                                   tag=f"p{'CD'[blk_i % 2]}",
                                   name=f"ctx_{h}_{qb}")
                # fp16 exp tiles and their pairwise-reduction tree (DVE)
                t1 = [None] * 8
                t2 = [None] * 4
                t3 = [None] * 2
                es = [None]
                etiles = [None] * KT

                for kt in range(KT):
                    g = blk_i * KT + kt
                    if kt == 2 and blk_i - 1 in sum_jobs:
                        sum_jobs.pop(blk_i - 1)()
                    s_ps = psum.tile([128, TBS], F32, tag=f"p{'ABH'[g % 3]}",
                                     name=f"s_{h}_{qb}_{kt}")
                    nc.tensor.matmul(s_ps[:], kT[:, kt * 128:(kt + 1) * 128],
                                     qT[:, h, qsl], start=True, stop=True)
                    e = blk.tile([128, TBS], F16, tag="blk",
                                 name=f"e_{h}_{qb}_{kt}")
                    nc.scalar.activation(e[:], s_ps[:],
                                         mybir.ActivationFunctionType.Exp,
                                         scale=SCALE, bias=nln16[:])
                    etiles[kt] = e
                    if kt % 2 == 1:
                        j = kt // 2
                        t1[j] = scratch.tile([128, TBS], F16, tag="t1", bufs=8,
                                             name=f"t1_{blk_i}_{j}")
                        nc.vector.tensor_add(t1[j][:], etiles[kt - 1][:], e[:])
                    if kt % 4 == 3:
                        j = kt // 4
                        t2[j] = scratch.tile([128, TBS], F16, tag="t2", bufs=4,
                                             name=f"t2_{blk_i}_{j}")
                        nc.vector.tensor_add(t2[j][:], t1[2 * j][:],
                                             t1[2 * j + 1][:])
                    if kt % 8 == 7:
                        j = kt // 8
                        t3[j] = scratch.tile([128, TBS], F16, tag="t3", bufs=2,
                                             name=f"t3_{blk_i}_{j}")
                        nc.vector.tensor_add(t3[j][:], t2[2 * j][:],
                                             t2[2 * j + 1][:])
                    if kt == KT - 1:
                        es[0] = scratch.tile([128, TBS], F16, tag="es", bufs=2,
                                             name=f"es_{blk_i}")
                        nc.vector.tensor_add(es[0][:], t3[0][:], t3[1][:])
                    pend.append((kt, e, ctx_ps, blk_i))
                    if len(pend) > STAGGER:
                        flush_av()
                    if qb > 0 and kt in (5, 9):
                        flush_wo(2)

                def sum_job(blk_i=blk_i, es=es):
                    sum_ps = psum.tile([1, TBS], F32, tag="pG", bufs=1,
                                       name=f"sum_{blk_i}")
                    nc.tensor.matmul(sum_ps[:], ones_col[:], es[0][:],
                                     start=True, stop=True)
                    recip = rows.tile([1, TBS], F32, tag="recip",
                                      name=f"recip_{blk_i}")
                    nc.vector.reciprocal_approx_fast(out=recip[:],
                                                     in_=sum_ps[:])
                    recipb = scratch.tile([128, TBS], F32, tag="bcast",
                                          bufs=2, name=f"recipb_{blk_i}")
                    nc.gpsimd.partition_broadcast(recipb[:], recip[:])
                    sum_jobs[blk_i] = ("done", recipb)
                    return recipb
                sum_jobs[blk_i] = lambda blk_i=blk_i, es=es: sum_job(blk_i, es)

                def norm_job(h=h, qb=qb, qsl=qsl, ctx_ps=ctx_ps, blk_i=blk_i):
                    ent = sum_jobs.pop(blk_i, None)
                    if callable(ent):
                        recipb = ent()
                        sum_jobs.pop(blk_i, None)
                    else:
                        recipb = ent[1]
                    nc.vector.tensor_mul(ctxT[:, h, qsl], ctx_ps[:],
                                         recipb[:])
                norm_jobs[blk_i] = norm_job
            pending_wo.extend(emit_wo(qb))

        while pend:
            flush_av()
        for i in sorted(list(norm_jobs)):
            norm_jobs.pop(i)()
        flush_wo(len(pending_wo))

    nc.compile()
    return nc


def _prep_inputs(hidden_states, positions, Wq, Wk, Wv, Wo, q_norm_w, k_norm_w):
    hidden_states = np.asarray(hidden_states, dtype=np.float32)
    positions = np.asarray(positions)
    Wq = np.asarray(Wq, dtype=np.float32)
    Wk = np.asarray(Wk, dtype=np.float32)
    Wv = np.asarray(Wv, dtype=np.float32)
    Wo = np.asarray(Wo, dtype=np.float32)
    q_norm_w = np.asarray(q_norm_w, dtype=np.float32)
    k_norm_w = np.asarray(k_norm_w, dtype=np.float32)

    inv_freq = THETA ** (-np.arange(HALF, dtype=np.float32) / HALF)
    perm_m = np.zeros((128, 128), dtype=np.float32)
    perm_m[np.arange(128), (np.arange(128) + HALF) % 128] = 1

    def tile_p(a, nt, w):
        # [nt*128, w] -> [128, nt, w]
        return np.ascontiguousarray(
            a.reshape(nt, 128, w).transpose(1, 0, 2)).astype(F16_NP)

    in_maps = []
    for c in range(DP * TP):
        b, g = divmod(c, TP)
        freqs = positions[b].astype(np.float32)[:, None] * inv_freq[None, :]
        cos = np.cos(freqs).T.astype(np.float32)      # [64, S]
        sin = np.sin(freqs).T.astype(np.float32)
        cos2 = np.ascontiguousarray(np.concatenate([cos, cos], axis=0))
        sin2 = np.ascontiguousarray(np.concatenate([-sin, sin], axis=0))
        in_maps.append({
            "xt": tile_p(hidden_states[b].T, HT, S),
            "wq": tile_p(Wq[:, g * DQ:(g + 1) * DQ], HT, DQ),
            "wk": tile_p(Wk[:, g * D:(g + 1) * D], HT, D),
            "wv": tile_p(Wv[:, g * D:(g + 1) * D], HT, D),
            "wo": tile_p(Wo[g * DQ:(g + 1) * DQ, :], HG, HID),
            "cos2": cos2,
            "sin2": sin2,
            "qnw": np.ascontiguousarray(q_norm_w[:, None]),
            "knw": np.ascontiguousarray(k_norm_w[:, None]),
            "iden": np.eye(128, dtype=F16_NP),
            "onesm": np.ones((128, 128), dtype=F16_NP),
            "perm": perm_m,
            "ones": np.ones((128, 1), dtype=F16_NP),
        })
    return in_maps


def _run(inputs, trace=False):
    skip_w = bool(np.allclose(inputs["q_norm_w"], 1.0)
                  and np.allclose(inputs["k_norm_w"], 1.0))
    key = ("nc", skip_w)
    if key not in _cache:
        _cache[key] = _build(skip_w)
    nc = _cache[key]
    in_maps = _prep_inputs(**inputs)
    res = run_bass_kernel_spmd(nc, in_maps, core_ids=list(range(DP * TP)),
                               trace=trace)
    out = np.zeros((B, S, HID), dtype=np.float32)
    for c in range(DP * TP):
        out[c // TP] += res.results[c]["out"]
    return out, res


def kernel(**inputs):
    out, _ = _run(inputs, trace=False)
    return out
